# revision 18
# baseline (speedup 1.0000x reference)
"""Trainium2 Bass kernel for nn_ConceptDescribe: bidirectional masked LSTM.

Sharding: data-parallel over batch across 8 NeuronCores (B=256 -> 32/core),
both directions computed on every core.

Device layout (per core):
  z PSUM [128, 512]: partition strips of 32 = (dir, H-half) col groups,
  free dim = [i|f|o|g] x 128 cols (host-permuted weight columns).
  Keras-style masking is folded into the gates by a rank-2 matmul pass that
  adds +/-BIG to masked rows' pre-activations (sigmoid saturates exactly to
  0/1 in bf16), plus a per-partition (1-m) blend for the h state.
"""

import sys

sys.path.insert(0, "/opt/trn_rl_repo")

import numpy as np
import ml_dtypes

import concourse.bass as bass
import concourse.bacc as bacc
import concourse.tile as tile
from concourse import mybir
from concourse.bass_utils import run_bass_kernel_spmd

BF16 = mybir.dt.bfloat16
F32 = mybir.dt.float32
NPBF16 = ml_dtypes.bfloat16

# Problem shapes (hardcoded per harness contract)
CHAR_SIZE, EMBED, HID = 20000, 128, 256
B, T = 256, 512
NCORES = 8
BC = B // NCORES  # 32 batch rows per core
NG = 4  # col groups: (dir, H-half)
BIG = 30.0  # gate saturation offset for masked steps

_MOD_CACHE = {}
LAST_RES = None  # BassKernelResults of the most recent kernel() device run
TRACE = False  # set True (e.g. from test.py) to capture an NTFF profile
TMODE = "pe"  # h-transpose route: "dma" | "pe"
ESPLIT = True  # per-direction elementwise chains
# engine per op: "v"=vector(DVE), "g"=gpsimd, "s"=scalar(ACT)
ASSIGN = {"tcopy": "v"}
DIRENG = {0: "v", 1: "g"}  # which engine runs each direction's elementwise
TCOPY = {0: "v", 1: "s"}  # hT PSUM->SBUF copy engine per direction


def _gate_perm():
    """Column permutation of the 4H=1024 gate dim.

    Original z cols: i=[0,256) f=[256,512) g=[512,768) o=[768,1024).
    New layout, per H-half kappa: block kappa (512 wide) = [i_k|f_k|o_k|g_k],
    each 128 wide, where i_k = i[128k:128k+128] etc.
    """
    perm = []
    for k in range(2):
        for gate in (0, 1, 3, 2):  # i, f, o, g
            base = 256 * gate + 128 * k
            perm.extend(range(base, base + 128))
    return np.array(perm, dtype=np.int64)


def _mask_pattern():
    """Row added (scaled by 1-m) to z for masked steps: +BIG on f cols,
    -BIG on i, o, g cols. Shape [1024] in permuted order."""
    pat = np.empty(1024, np.float32)
    for k in range(2):
        o = 512 * k
        pat[o + 0 : o + 128] = -BIG  # i
        pat[o + 128 : o + 256] = +BIG  # f
        pat[o + 256 : o + 384] = -BIG  # o
        pat[o + 384 : o + 512] = -BIG  # g
    return pat


def _build_module(steps, ablate=(), has_bias=False, run_steps=None):
    """Build the Bass module (shared across all 8 cores, SPMD).

    ablate: subset of {"hmm", "xmm", "maskmm", "act", "elem", "trans"} — skip
    those parts (correctness-breaking; for cost attribution only).
    run_steps: loop only this many steps while keeping input shapes sized
    for `steps` (timing experiments; wrong results for run_steps < steps).
    """
    nc = bacc.Bacc()
    if run_steps is None:
        run_steps = steps

    TB = steps * BC
    # DRAM inputs (per-core shards / replicated weights)
    xtf_d = nc.dram_tensor("xtf", [EMBED, TB], BF16, kind="ExternalInput")
    xtb_d = nc.dram_tensor("xtb", [EMBED, TB], BF16, kind="ExternalInput")
    mb_d = nc.dram_tensor("mb", [2, 2 * TB], BF16, kind="ExternalInput")
    m1m_d = nc.dram_tensor("m1m", [128, 2 * steps], F32, kind="ExternalInput")
    wf_d = nc.dram_tensor("wf", [EMBED, 1024], BF16, kind="ExternalInput")
    wb_d = nc.dram_tensor("wb", [EMBED, 1024], BF16, kind="ExternalInput")
    uf0_d = nc.dram_tensor("uf0", [128, 1024], BF16, kind="ExternalInput")
    uf1_d = nc.dram_tensor("uf1", [128, 1024], BF16, kind="ExternalInput")
    ub0_d = nc.dram_tensor("ub0", [128, 1024], BF16, kind="ExternalInput")
    ub1_d = nc.dram_tensor("ub1", [128, 1024], BF16, kind="ExternalInput")
    pat_d = nc.dram_tensor("pat", [2, 2048], BF16, kind="ExternalInput")
    ident_d = nc.dram_tensor("ident", [128, 192], BF16, kind="ExternalInput")
    hout_d = nc.dram_tensor("hout", [128, 128], BF16, kind="ExternalOutput")

    SIG = mybir.ActivationFunctionType.Sigmoid
    TANH = mybir.ActivationFunctionType.Tanh
    MUL = mybir.AluOpType.mult
    ADD = mybir.AluOpType.add

    with tile.TileContext(nc) as tc:
        with (
            tc.tile_pool(name="const", bufs=1) as const,
            tc.tile_pool(name="act", bufs=3) as actp,
            tc.tile_pool(name="tmp", bufs=4) as tmp,
            tc.tile_pool(name="state", bufs=2) as statep,
            tc.tile_pool(name="ht", bufs=3) as htp,
            tc.tile_pool(name="psum", bufs=2, space="PSUM") as psum,
        ):
            # --- load everything resident into SBUF ---
            xtf = const.tile([EMBED, TB], BF16)
            xtb = const.tile([EMBED, TB], BF16)
            mb = const.tile([2, 2 * TB], BF16)
            m1m = const.tile([128, 2 * steps], F32)
            wf = const.tile([EMBED, 1024], BF16)
            wb = const.tile([EMBED, 1024], BF16)
            uf0 = const.tile([128, 1024], BF16)
            uf1 = const.tile([128, 1024], BF16)
            ub0 = const.tile([128, 1024], BF16)
            ub1 = const.tile([128, 1024], BF16)
            pat = const.tile([2, 2048], BF16)
            ident = const.tile([128, 192], BF16)
            for sb_t, dr in (
                (xtf, xtf_d), (xtb, xtb_d), (mb, mb_d), (m1m, m1m_d),
                (wf, wf_d), (wb, wb_d), (uf0, uf0_d), (uf1, uf1_d),
                (ub0, ub0_d), (ub1, ub1_d), (pat, pat_d), (ident, ident_d),
            ):
                nc.sync.dma_start(sb_t[:], dr[:])

            W = (wf, wb)
            U = ((uf0, uf1), (ub0, ub1))
            XT = (xtf, xtb)

            # initial states (bf16 zeros)
            h_prev = statep.tile([128, 128], BF16, tag="h")
            c_prev = statep.tile([128, 128], BF16, tag="c")
            nc.vector.memset(h_prev[:], 0.0)
            nc.gpsimd.memset(c_prev[:], 0.0)

            hT = None  # [128, 64] per dir tiles, set at end of each step

            for t in range(run_steps):
                z = psum.tile([128, 512], F32, tag="z")
                # --- matmul passes: 4 col groups = (dir d, H-half kappa) ---
                for d in range(2):
                    for k in range(2):
                        g = 2 * d + k
                        zs = z[32 * g : 32 * (g + 1), :]
                        tp = (0, 32 * g)
                        rhs_cols = slice(512 * k, 512 * (k + 1))
                        passes = []
                        if "xmm" not in ablate:
                            passes.append((XT[d][:, t * BC : (t + 1) * BC],
                                           W[d][:, rhs_cols]))
                        if "maskmm" not in ablate:
                            # rank-2: (1-m) row x (+-BIG mask pattern) + ones x b
                            passes.append((
                                mb[:, (d * steps + t) * BC
                                   : (d * steps + t + 1) * BC],
                                pat[:, 1024 * d + rhs_cols.start
                                    : 1024 * d + rhs_cols.stop],
                            ))
                        if t > 0 and "hmm" not in ablate and "trans" not in ablate:
                            passes.append((hT[d][:, 0:32], U[d][0][:, rhs_cols]))
                            passes.append((hT[d][:, 32:64], U[d][1][:, rhs_cols]))
                        for ip, (lhsT, rhs) in enumerate(passes):
                            nc.tensor.matmul(
                                zs, lhsT, rhs,
                                start=(ip == 0), stop=(ip == len(passes) - 1),
                                tile_position=tp,
                            )

                # --- activations (fused across dirs) ---
                ENG = {"v": nc.vector, "g": nc.gpsimd, "s": nc.scalar}
                A = actp.tile([128, 384], BF16, tag="A")  # sigma(i,f,o)
                TG = actp.tile([128, 128], BF16, tag="TG")  # tanh(g)
                TC = actp.tile([128, 128], BF16, tag="TC")
                q = tmp.tile([128, 128], BF16, tag="q")
                r = tmp.tile([128, 128], BF16, tag="r")
                h1 = tmp.tile([128, 128], BF16, tag="h1")
                c_new = statep.tile([128, 128], BF16, tag="c")
                h_new = statep.tile([128, 128], BF16, tag="h")
                new_hT = [None, None]
                if "act" not in ablate:
                    nc.scalar.activation(A[:], z[:, 0:384], SIG)
                    nc.scalar.activation(TG[:], z[:, 384:512], TANH)
                else:
                    nc.scalar.activation(A[:, 0:1], z[:, 0:1], SIG)
                    nc.scalar.activation(TG[:, 0:1], z[:, 1:2], TANH)

                dslices = [slice(0, 64), slice(64, 128)] if ESPLIT \
                    else [slice(0, 128)]
                for ds in dslices:
                    d0 = 0 if ds.start == 0 else 1
                    E = ENG[DIRENG[d0]] if ESPLIT else ENG["v"]
                    m1_t = m1m[ds, 2 * t : 2 * t + 1]  # 1-m
                    si = A[ds, 0:128]
                    sf = A[ds, 128:256]
                    so = A[ds, 256:384]

                    if "elem" not in ablate:
                        # masked rows have sf=1, si=so=0 via the +-BIG pass:
                        # c = sf*c + si*tg is exact masking already
                        E.tensor_tensor(
                            out=q[ds, :], in0=TG[ds, :], in1=si, op=MUL)
                        E.tensor_tensor(
                            out=r[ds, :], in0=sf, in1=c_prev[ds, :], op=MUL)
                        E.tensor_tensor(
                            out=c_new[ds, :], in0=r[ds, :], in1=q[ds, :],
                            op=ADD)

                        # h = (1-m)*h_prev + sigma(o)*tanh(c)  [sigma(o)=0 when
                        # masked, so the h1 term vanishes exactly]
                        nc.scalar.activation(TC[ds, :], c_new[ds, :], TANH)
                        E.tensor_tensor(
                            out=h1[ds, :], in0=TC[ds, :], in1=so, op=MUL)
                        nc.vector.scalar_tensor_tensor(
                            out=h_new[ds, :], in0=h_prev[ds, :],
                            scalar=m1_t, in1=h1[ds, :], op0=MUL, op1=ADD)
                        hsrc = h_new
                    else:
                        hsrc = h_prev

                    if t < run_steps - 1 and "trans" not in ablate:
                        npart = ds.stop - ds.start
                        if TMODE == "pe":
                            sfx = str(d0) if ESPLIT else ""
                            hTp = psum.tile([128, npart], BF16, tag="hTp" + sfx)
                            idslice = (ident[ds, 128:192] if npart == 64
                                       else ident[:, 0:128])
                            nc.tensor.transpose(hTp[:], hsrc[ds, :], idslice)
                            hTs = htp.tile([128, npart], BF16, tag="hTs" + sfx)
                            tce = TCOPY[d0] if ESPLIT else ASSIGN["tcopy"]
                            if tce == "s":
                                nc.scalar.copy(hTs[:], hTp[:])
                            else:
                                ENG[tce].tensor_copy(hTs[:], hTp[:])
                            if ESPLIT:
                                new_hT[d0] = hTs
                            else:
                                new_hT = [hTs[:, 0:64], hTs[:, 64:128]]
                        else:
                            if ESPLIT:
                                hTd = htp.tile([128, 64], BF16, tag=f"hTd{d0}")
                                dmae = nc.sync if d0 == 0 else nc.scalar
                                dmae.dma_start_transpose(hTd[:], hsrc[ds, :])
                                new_hT[d0] = hTd
                            else:
                                hTf = htp.tile([128, 64], BF16, tag="hTf")
                                hTb = htp.tile([128, 64], BF16, tag="hTb")
                                nc.sync.dma_start_transpose(hTf[:], hsrc[0:64, :])
                                nc.scalar.dma_start_transpose(
                                    hTb[:], hsrc[64:128, :])
                                new_hT = [hTf, hTb]

                if "elem" not in ablate:
                    c_prev = c_new
                    h_prev = h_new
                if t < run_steps - 1 and "trans" not in ablate:
                    hT = tuple(new_hT)

            nc.sync.dma_start(hout_d[:], h_prev[:])

    nc.compile()
    return nc


def _host_prep(inputs, input_end, embed_table, Wf, Uf, bf, Wb, Ub, bb, steps=T):
    """Shard + lay out inputs for the 8 cores. Returns in_maps list."""
    perm = _gate_perm()
    maskpat = _mask_pattern()
    TT_ = steps

    inputs = np.asarray(inputs)[:, :steps]
    mask = (inputs > 0)  # [B, steps] bool
    embed = np.asarray(embed_table, np.float32)[inputs]  # [B,steps,E]

    Wp = (np.asarray(Wf, np.float32)[:, perm], np.asarray(Wb, np.float32)[:, perm])
    Up = (np.asarray(Uf, np.float32)[:, perm], np.asarray(Ub, np.float32)[:, perm])
    bp = (np.asarray(bf, np.float32)[perm], np.asarray(bb, np.float32)[perm])

    wf16 = Wp[0].astype(NPBF16)
    wb16 = Wp[1].astype(NPBF16)
    uf0 = Up[0][0:128].astype(NPBF16)
    uf1 = Up[0][128:256].astype(NPBF16)
    ub0 = Up[1][0:128].astype(NPBF16)
    ub1 = Up[1][128:256].astype(NPBF16)

    # pat rows: row0 = mask pattern, row1 = bias; per dir side by side
    pat = np.zeros((2, 2048), np.float32)
    pat[0, 0:1024] = maskpat
    pat[0, 1024:2048] = maskpat
    pat[1, 0:1024] = bp[0]
    pat[1, 1024:2048] = bp[1]
    pat16 = pat.astype(NPBF16)

    ident_np = np.zeros((128, 192), NPBF16)
    ident_np[:, 0:128] = np.eye(128, dtype=NPBF16)
    ident_np[0:64, 128:192] = np.eye(64, dtype=NPBF16)
    ident_np[64:128, 128:192] = np.eye(64, dtype=NPBF16)

    in_maps = []
    for cc in range(NCORES):
        bs = slice(cc * BC, (cc + 1) * BC)
        emb_c = embed[bs]  # [32, TT_, 128]
        m_c = mask[bs]  # [32, TT_]

        # xT: [E, TT_*BC], xtf[e, t*BC+b] = emb_c[b, t, e]
        xtf = np.ascontiguousarray(emb_c.transpose(2, 1, 0).reshape(EMBED, TT_ * BC))
        xtb = np.ascontiguousarray(
            emb_c[:, ::-1].transpose(2, 1, 0).reshape(EMBED, TT_ * BC)
        )

        # mb [2, 2*TT_*BC]: row0 = (1-m), row1 = 1; fwd then bwd halves
        one_m_f = (~m_c).T.reshape(TT_ * BC).astype(np.float32)  # [t*BC+b]
        one_m_b = (~m_c[:, ::-1]).T.reshape(TT_ * BC).astype(np.float32)
        mbm = np.ones((2, 2 * TT_ * BC), np.float32)
        mbm[0, 0 : TT_ * BC] = one_m_f
        mbm[0, TT_ * BC :] = one_m_b

        # m1m [128, 2*TT_]: cols 2t = (1-m), 2t+1 = m; group-layout rows
        m1m = np.empty((128, 2 * TT_), np.float32)
        m1m[0:32, 0::2] = one_m_f.reshape(TT_, BC).T
        m1m[64:96, 0::2] = one_m_b.reshape(TT_, BC).T
        m1m[32:64, 0::2] = m1m[0:32, 0::2]
        m1m[96:128, 0::2] = m1m[64:96, 0::2]
        m1m[:, 1::2] = 1.0 - m1m[:, 0::2]

        in_maps.append(
            {
                "xtf": xtf.astype(NPBF16),
                "xtb": xtb.astype(NPBF16),
                "mb": mbm.astype(NPBF16),
                "m1m": m1m,
                "wf": wf16, "wb": wb16,
                "uf0": uf0, "uf1": uf1, "ub0": ub0, "ub1": ub1,
                "pat": pat16, "ident": ident_np,
            }
        )
    return in_maps


def _unpack_h(houts):
    """houts: list of [128,128] bf16 per core -> bivalue [B, 2H] f32."""
    bivalue = np.empty((B, 2 * HID), np.float32)
    for cc, hb in enumerate(houts):
        hb = np.asarray(hb, np.float32)
        bs = slice(cc * BC, (cc + 1) * BC)
        for k in range(2):
            # h_f[b, 128k+e] = hb[32k+b, e]
            bivalue[bs, 128 * k : 128 * (k + 1)] = hb[32 * k : 32 * k + 32]
            bivalue[bs, HID + 128 * k : HID + 128 * (k + 1)] = hb[
                64 + 32 * k : 64 + 32 * k + 32
            ]
    return bivalue


def run_device(inputs, input_end, embed_table, Wf, Uf, bf, Wb, Ub, bb,
               steps=T, trace=False, **kw):
    """Build+run the device part; returns (bivalue [B,2H] f32, BassKernelResults)."""
    has_bias = bool(np.any(np.asarray(bf)) or np.any(np.asarray(bb)))
    key = (steps, has_bias)
    if key not in _MOD_CACHE:
        _MOD_CACHE[key] = _build_module(steps, has_bias=has_bias)
    nc = _MOD_CACHE[key]
    in_maps = _host_prep(inputs, input_end, embed_table, Wf, Uf, bf, Wb, Ub, bb,
                         steps=steps)
    res = run_bass_kernel_spmd(
        nc, in_maps, core_ids=list(range(NCORES)), trace=trace, **kw
    )
    bivalue = _unpack_h([r["hout"] for r in res.results])
    return bivalue, res


def _softmax(x):
    e = np.exp(x - x.max(axis=-1, keepdims=True))
    return e / e.sum(axis=-1, keepdims=True)


def kernel(inputs, input_end, embed_table, Wf, Uf, bf, Wb, Ub, bb,
           cls_W, cls_b, ent_W, ent_b):
    global LAST_RES
    inputs = np.asarray(inputs)
    bivalue, LAST_RES = run_device(
        inputs, input_end, embed_table, Wf, Uf, bf, Wb, Ub, bb, trace=TRACE
    )
    sentence_end = bivalue[np.asarray(input_end)]  # [B, 2H]
    cls = 1.0 / (1.0 + np.exp(-(sentence_end @ np.asarray(cls_W, np.float32)
                                + np.asarray(cls_b, np.float32))))
    ent = _softmax(bivalue @ np.asarray(ent_W, np.float32)
                   + np.asarray(ent_b, np.float32))
    return cls.astype(np.float32), ent.astype(np.float32)
